# revision 1
# baseline (speedup 1.0000x reference)
"""Trainium2 Bass kernel for nn_DualAttention (S=2048, B=16, H2=2048, V=1024).

Computation (per the reference):
    sum_w = hidden @ Ww + bw + z @ Wz + bz + w_a*0.5        [S, B, V]
    u     = tanh(sum_w) @ Vw + vb                            [S, B, 1]
    out   = softmax(u, axis=0)                               [S, B, 1]

Strategy
--------
Data-parallel over batch: 16 batches -> 2 per NeuronCore (8 cores).
Host-side prep per core:
  * concat hidden/z along the hidden axis -> X [ROWS=4096, H=4096]
    (rows are b-major: row = b_local*2048 + s)
  * transpose to xt = X^T [H, ROWS], cast to the matmul dtype
  * W = concat([Ww, Wz], 0) [H, V], reordered into per-(vb,k) 128x128
    tiles; bias = bw + bz + 0.5*w_a
Device kernel (per core), W-stationary matmul with psum layout [v, rows]:
  for each rowblock (RB rows):
    load xt[:, rowblock] into SBUF (one [128, RB] tile per k)
    for vb in 0..7:                       # 128-wide slices of V
      psum[vb] += sum_k W[vb,k].T @ xt[k]      (32 accumulating matmuls)
      t = tanh(psum + bias_vb)            # one ACT op, per-partition bias
      u_psum += Vw[vb].T @ t              # [1, RB] second-stage matmul (f32r)
    u_scratch[rowblock] = u_psum          # via SBUF bounce -> DRAM
  softmax over s per batch (no max subtraction: u is tanh-bounded):
    DMA u_scratch -> [2, 2048], exp+rowsum on ACT (in place),
    reciprocal + scale on DVE (in place), DMA out [2, 2048].

The vb scalar is dropped: softmax is shift-invariant.

MAIN_DT selects the matmul dtype: "bf16" (faster, ~1e-2 rel err) or
"f32r" (fp32 data with the PE's fast rounded-fp32 mode, ~1e-3 rel err).
"""

import numpy as np
import ml_dtypes

# ---------------------------------------------------------------------------
# Problem constants (hardcoded; kernel.py must be self-contained)
# ---------------------------------------------------------------------------
S, B, H2, V = 2048, 16, 2048, 1024
ALPHA_S = 0.5
NCORES = 8
BC = B // NCORES            # local batches per core
ROWS = S * BC               # 4096 rows per core (b-major)
H = 2 * H2                  # 4096 contraction dim (hidden ++ z)
P = 128
NK = H // P                 # 32
NVB = V // P                # 8

MAIN_DT = "f32r"            # "bf16" | "f32r"
RB = 512 if MAIN_DT == "bf16" else 256
NRB = ROWS // RB


# ---------------------------------------------------------------------------
# Workarounds for this walrus build's 1-sync-wait-per-instruction limit
# ---------------------------------------------------------------------------
def _install_drain_patch():
    import concourse.mybir as mybir
    from concourse.tile import TileContext
    from concourse.vector_clock import ScopedClock

    def _drain_and_barrier(self, tick_clock, wait_clock):
        nc = self.nc
        drain_inst = nc.sync.drain()
        wait_clock.add_sem_waits(
            drain_inst.ins, ScopedClock({None: tick_clock.global_clock})
        )
        si = drain_inst.ins.sync_info
        if si is not None:
            waits = list(si.on_wait)
            if len(waits) > 1:
                si.on_wait = [waits[0]]
                for w in waits[1:]:
                    nop = nc.sync.nop(nofuse=True)
                    nop.ins.sync_info = mybir.SyncInfo(on_wait=[w], on_update=[])
        nc.all_engine_barrier()
        assert self.sems is not None
        popped = nc._tile_sem_poison_stack.pop()
        assert popped is self._sem_poison
        nc.clear_and_free_semaphores(list(self.sems.allocated().values()))
        nc.all_engine_barrier()

    TileContext._drain_and_barrier = _drain_and_barrier


def _split_multiwait(nc):
    """Hoist extra sync waits onto same-engine event-semaphore instructions
    inserted just before the carrying instruction."""
    import concourse.mybir as mybir

    counter = 0
    for fn in nc.m.functions:
        for bb in fn.blocks:
            insts = bb.instructions
            new_list = []
            changed = False
            for inst in insts:
                si = inst.sync_info
                if si is not None:
                    waits = list(si.on_wait)
                    if len(waits) > 1:
                        for w in waits[:-1]:
                            counter += 1
                            nop = mybir.InstEventSemaphore(
                                name=f"I-mwsplit-{counter}"
                            )
                            nop.engine = inst.engine
                            nop.bass_nofuse = True
                            nop.sync_info = mybir.SyncInfo(
                                on_wait=[w], on_update=[]
                            )
                            nc.register_instruction(nop)
                            new_list.append(nop)
                        si.on_wait = [waits[-1]]
                        changed = True
                new_list.append(inst)
            if changed:
                bb.instructions = new_list
    return counter


# ---------------------------------------------------------------------------
# Kernel build
# ---------------------------------------------------------------------------
def _build_nc():
    import concourse.bass as bass
    import concourse.mybir as mybir
    from concourse.tile import TileContext

    f32 = mybir.dt.float32
    f32r = mybir.dt.float32r
    DT = mybir.dt.bfloat16 if MAIN_DT == "bf16" else f32r

    nc = bass.Bass()
    # W pre-tiled host-side: tile (vb, k) is [P, 128] contiguous
    w_d = nc.declare_dram_parameter("w", [NVB, P, NK * P], DT, isOutput=False)
    xt_d = nc.declare_dram_parameter("xt", [H, ROWS], DT, isOutput=False)
    bct_d = nc.declare_dram_parameter("bct", [P, NVB], f32, isOutput=False)
    vwt_d = nc.declare_dram_parameter("vwt", [P, NVB], f32r, isOutput=False)
    att_d = nc.declare_dram_parameter("att", [BC, S], f32, isOutput=True)

    u_scr = nc.dram_tensor("u_scr", [ROWS], f32)

    with TileContext(nc) as tc:
        with (
            tc.tile_pool(name="wpool", bufs=1) as wpool,
            tc.tile_pool(name="xpool", bufs=1) as xpool,
            tc.tile_pool(name="tpool", bufs=1) as tpool,
            tc.tile_pool(name="spool", bufs=1) as spool,
            tc.tile_pool(name="pspool", bufs=1, space="PSUM") as pspool,
        ):
            # --- constants ---
            bct_sb = spool.tile([P, NVB], f32, name="bct_sb")
            nc.sync.dma_start(out=bct_sb[:], in_=bct_d[:, :])
            vwt_sb = spool.tile([P, NVB], f32r, name="vwt_sb")
            nc.sync.dma_start(out=vwt_sb[:], in_=vwt_d[:, :])

            # --- resident weights: vb0's tiles first (fast start), then rest
            # each vb's weights may be split into `nsplit` tiles along k so
            # the first matmuls can start before the whole slab lands
            w_sb = [None] * NVB

            def load_w(vb, nsplit=1):
                kc = NK // nsplit
                tiles = []
                for j in range(nsplit):
                    t = wpool.tile([P, kc, P], DT, name=f"w_{vb}_{j}")
                    nc.sync.dma_start(
                        out=t[:],
                        in_=w_d[vb, :, j * kc * P : (j + 1) * kc * P].rearrange(
                            "p (k q) -> p k q", q=P
                        ),
                    )
                    tiles.append(t)
                w_sb[vb] = (tiles, kc)

            def w_tile(vb, k):
                tiles, kc = w_sb[vb]
                return tiles[k // kc][:, k % kc]


            # xt loaded in groups of KG k-tiles (>=1 MiB per DMA)
            KG = 8
            NKG = NK // KG
            xt_r = xt_d[:, :].rearrange(
                "(g q p) (r c) -> p r g q c", p=P, q=KG, c=RB
            )

            def load_xt(r):
                tiles = []
                for g in range(NKG):
                    t = xpool.tile(
                        [P, KG, RB], DT, name=f"xt_{r}_{g}", tag="xt",
                        bufs=2 * NKG,
                    )
                    nc.sync.dma_start(out=t[:], in_=xt_r[:, r, g])
                    tiles.append(t)
                return tiles

            load_w(0, nsplit=8)
            xt_tiles = load_xt(0)
            for vb in range(1, NVB):
                load_w(vb)

            for r in range(NRB):
                u_ps = pspool.tile([1, RB], f32, name="u_ps", tag="ups", bufs=2)
                for vb in range(NVB):
                    ps = pspool.tile([P, RB], f32, name="ps", tag="ps", bufs=2)
                    for k in range(NK):
                        nc.tensor.matmul(
                            ps[:],
                            w_tile(vb, k),
                            xt_tiles[k // KG][:, k % KG],
                            start=(k == 0),
                            stop=(k == NK - 1),
                        )
                    tt = tpool.tile([P, RB], f32r, name="tt", tag="tt", bufs=2)
                    nc.scalar.activation(
                        tt[:],
                        ps[:],
                        mybir.ActivationFunctionType.Tanh,
                        bias=bct_sb[:, vb : vb + 1],
                        scale=1.0,
                    )
                    nc.tensor.matmul(
                        u_ps[:],
                        vwt_sb[:, vb : vb + 1],
                        tt[:],
                        start=(vb == 0),
                        stop=(vb == NVB - 1),
                    )
                if r + 1 < NRB:
                    xt_tiles = load_xt(r + 1)
                u_sb = spool.tile([1, RB], f32, name="u_sb", tag="usb", bufs=2)
                nc.vector.tensor_copy(u_sb[:], u_ps[:])
                nc.sync.dma_start(
                    out=u_scr[r * RB : (r + 1) * RB], in_=u_sb[:]
                )

            # --- softmax over s per local batch ---
            u2 = spool.tile([BC, S], f32, name="u2")
            nc.sync.dma_start(
                out=u2[:], in_=u_scr[:].rearrange("(b s) -> b s", b=BC)
            )
            esum = spool.tile([BC, 1], f32, name="esum")
            nc.scalar.activation(
                u2[:],
                u2[:],
                mybir.ActivationFunctionType.Exp,
                accum_out=esum[:],
            )
            rec = spool.tile([BC, 1], f32, name="rec")
            nc.vector.reciprocal(rec[:], esum[:])
            nc.vector.tensor_scalar_mul(u2[:], u2[:], rec[:])
            nc.sync.dma_start(out=att_d[:, :], in_=u2[:])

    _split_multiwait(nc)
    return nc


# ---------------------------------------------------------------------------
# Host entry point
# ---------------------------------------------------------------------------
def kernel(hidden, z, Ww, bw, Wz, bz, Vw, vb, w_a):
    _install_drain_patch()
    from concourse.bass_utils import run_bass_kernel_spmd

    np_main = ml_dtypes.bfloat16 if MAIN_DT == "bf16" else np.float32

    # ---- host-side shard prep ----
    hid_t = np.ascontiguousarray(
        np.asarray(hidden).astype(np_main).transpose(2, 1, 0)
    )  # [H2, B, S]
    z_t = np.ascontiguousarray(
        np.asarray(z).astype(np_main).transpose(2, 1, 0)
    )  # [H2, B, S]

    w_cat = np.concatenate(
        [np.asarray(Ww), np.asarray(Wz)], axis=0
    ).astype(np_main)  # [H, V]
    # reorder so tile (vb) is [P, NK*P] with per-partition-contiguous rows:
    # w_r[vb, p, k*P+q] = W[k*P+p, vb*P+q]
    w_r = np.ascontiguousarray(
        w_cat.reshape(NK, P, NVB, P).transpose(2, 1, 0, 3)
    ).reshape(NVB, P, NK * P)

    bias = (
        np.asarray(bw).astype(np.float64)
        + np.asarray(bz).astype(np.float64)
        + float(np.asarray(w_a)) * ALPHA_S
    ).astype(np.float32)  # [V]
    bct = np.ascontiguousarray(bias.reshape(NVB, P).T)  # [P, NVB]
    vwt = np.ascontiguousarray(
        np.asarray(Vw).astype(np.float32).reshape(NVB, P).T
    )  # [P, NVB]

    in_maps = []
    for c in range(NCORES):
        xt_c = np.empty((H, ROWS), dtype=np_main)
        xt_c[:H2] = hid_t[:, 2 * c : 2 * c + 2, :].reshape(H2, ROWS)
        xt_c[H2:] = z_t[:, 2 * c : 2 * c + 2, :].reshape(H2, ROWS)
        in_maps.append({"xt": xt_c, "w": w_r, "bct": bct, "vwt": vwt})

    nc = _build_nc()
    res = run_bass_kernel_spmd(nc, in_maps, list(range(NCORES)))

    out = np.empty((S, B, 1), dtype=np.float32)
    for c in range(NCORES):
        att = res.results[c]["att"]  # [BC, S]
        for b in range(BC):
            out[:, 2 * c + b, 0] = att[b]
    return out



# revision 2
# speedup vs baseline: 1.0219x; 1.0219x over previous
"""Trainium2 Bass kernel for nn_DualAttention (S=2048, B=16, H2=2048, V=1024).

Computation (per the reference):
    sum_w = hidden @ Ww + bw + z @ Wz + bz + w_a*0.5        [S, B, V]
    u     = tanh(sum_w) @ Vw + vb                            [S, B, 1]
    out   = softmax(u, axis=0)                               [S, B, 1]

Strategy
--------
Data-parallel over batch: 16 batches -> 2 per NeuronCore (8 cores).
Host-side prep per core:
  * concat hidden/z along the hidden axis -> X [ROWS=4096, H=4096]
    (rows are b-major: row = b_local*2048 + s), X^T pre-tiled to
    xt[r, p, k, c] = X^T[k*128+p, r*RB+c]  (contiguous per rowblock ->
    每 DMA is 128 partition lines of NK*RB*2B contiguous bytes)
  * W = concat([Ww, Wz], 0) [H, V] in bf16, tiled [NVB, P, NK*P]
  * bias = bw + bz + 0.5*w_a (f32), vwt = Vw columns (f32)
Device kernel (per core), W-stationary matmuls in bf16 (PE full rate),
f32 PSUM accumulate:
  for r in rowblocks (RB rows):
    for vb in 0..7:                       # 128-wide slices of V
      ps[128, RB] = sum_k W[vb,k].T @ xt[r,k]   (32 bf16 matmuls)
      tt = tanh(ps + bias_vb)             # ACT, per-partition bias
      acc = acc + tt * vwt_vb             # DVE scalar_tensor_tensor
    u_ps[1, RB] = ones.T @ acc            # ONE matmul (partition sum)
    u2[b, s-slice] = u_ps                 # copy into SBUF-resident u
  softmax over s per batch entirely in SBUF (u is tanh-bounded so no
  max subtraction): exp+rowsum on ACT, reciprocal + scale on DVE,
  DMA out [2, 2048].

The vb scalar is dropped: softmax is shift-invariant.

Startup: the first W slab (vb0) and the first xt rowblock are loaded
in 8 k-chunks each, dispatched on two independent DGE queues (sync +
scalar engines) so the first matmul starts ~4us in instead of ~20us.
Stage-2 (the V-contraction by Vw) runs on the otherwise-idle DVE, so
the PE only does the 4096 main matmuls plus 16 tiny partition-sum
matmuls. The ones-matmul for rowblock r is emitted in the middle of
rowblock r+1's stream so the PE never waits on ACT/DVE.
"""

import numpy as np
import ml_dtypes

# ---------------------------------------------------------------------------
# Problem constants (hardcoded; kernel.py must be self-contained)
# ---------------------------------------------------------------------------
S, B, H2, V = 2048, 16, 2048, 1024
ALPHA_S = 0.5
NCORES = 8
BC = B // NCORES            # local batches per core
ROWS = S * BC               # 4096 rows per core (b-major)
H = 2 * H2                  # 4096 contraction dim (hidden ++ z)
P = 128
NK = H // P                 # 32
NVB = V // P                # 8

MAIN_DT = "bf16"            # "bf16" | "f32r"
RB = 256
NRB = ROWS // RB            # 16
RB_PER_B = S // RB          # rowblocks per local batch

KC0 = 8                     # k-tiles per startup chunk (w0 / xt r0)
NCH0 = NK // KC0            # 4 startup chunks
KH = NK // 2                # steady-state xt half size (16 k-tiles)


# ---------------------------------------------------------------------------
# Workarounds for this walrus build's 1-sync-wait-per-instruction limit
# ---------------------------------------------------------------------------
def _install_drain_patch():
    import concourse.mybir as mybir
    from concourse.tile import TileContext
    from concourse.vector_clock import ScopedClock

    def _drain_and_barrier(self, tick_clock, wait_clock):
        nc = self.nc
        drain_inst = nc.sync.drain()
        wait_clock.add_sem_waits(
            drain_inst.ins, ScopedClock({None: tick_clock.global_clock})
        )
        si = drain_inst.ins.sync_info
        if si is not None:
            waits = list(si.on_wait)
            if len(waits) > 1:
                si.on_wait = [waits[0]]
                for w in waits[1:]:
                    nop = nc.sync.nop(nofuse=True)
                    nop.ins.sync_info = mybir.SyncInfo(on_wait=[w], on_update=[])
        nc.all_engine_barrier()
        assert self.sems is not None
        popped = nc._tile_sem_poison_stack.pop()
        assert popped is self._sem_poison
        nc.clear_and_free_semaphores(list(self.sems.allocated().values()))
        nc.all_engine_barrier()

    TileContext._drain_and_barrier = _drain_and_barrier


def _split_multiwait(nc):
    """Hoist extra sync waits onto same-engine event-semaphore instructions
    inserted just before the carrying instruction."""
    import concourse.mybir as mybir

    counter = 0
    for fn in nc.m.functions:
        for bb in fn.blocks:
            insts = bb.instructions
            new_list = []
            changed = False
            for inst in insts:
                si = inst.sync_info
                if si is not None:
                    waits = list(si.on_wait)
                    if len(waits) > 1:
                        for w in waits[:-1]:
                            counter += 1
                            nop = mybir.InstEventSemaphore(
                                name=f"I-mwsplit-{counter}"
                            )
                            nop.engine = inst.engine
                            nop.bass_nofuse = True
                            nop.sync_info = mybir.SyncInfo(
                                on_wait=[w], on_update=[]
                            )
                            nc.register_instruction(nop)
                            new_list.append(nop)
                        si.on_wait = [waits[-1]]
                        changed = True
                new_list.append(inst)
            if changed:
                bb.instructions = new_list
    return counter


# ---------------------------------------------------------------------------
# Kernel build
# ---------------------------------------------------------------------------
def _build_nc():
    import concourse.bass as bass
    import concourse.mybir as mybir
    from concourse.tile import TileContext

    f32 = mybir.dt.float32
    f32r = mybir.dt.float32r
    DT = mybir.dt.bfloat16 if MAIN_DT == "bf16" else f32r

    nc = bass.Bass()
    # W pre-tiled host-side: w[p, vb, k, q] = W[k*P+p, vb*P+q] — one SBUF
    # slab, loaded in NCH0 k-chunk DMAs each covering every vb
    w_d = nc.declare_dram_parameter("w", [P, NVB, NK, P], DT, isOutput=False)
    # xt pre-tiled host-side: [r, p, k, c] with (k, c) contiguous per row
    xt_d = nc.declare_dram_parameter("xt", [NRB, P, NK, RB], DT, isOutput=False)
    bct_d = nc.declare_dram_parameter("bct", [P, NVB], f32, isOutput=False)
    vwt_d = nc.declare_dram_parameter("vwt", [P, NVB], f32, isOutput=False)
    vwtr_d = nc.declare_dram_parameter("vwtr", [P, NVB], f32r, isOutput=False)
    ones_d = nc.declare_dram_parameter("ones", [P, 1], f32r, isOutput=False)
    att_d = nc.declare_dram_parameter("att", [BC, S], f32, isOutput=True)

    with TileContext(nc) as tc:
        with (
            tc.tile_pool(name="wpool", bufs=1) as wpool,
            tc.tile_pool(name="xpool", bufs=1) as xpool,
            tc.tile_pool(name="tpool", bufs=1) as tpool,
            tc.tile_pool(name="spool", bufs=1) as spool,
            tc.tile_pool(name="pspool", bufs=1, space="PSUM") as pspool,
        ):
            # --- resident weights: one slab, streamed as NCH0 k-chunks
            # (each chunk carries ALL vb for 8 k-tiles, matching rowblock
            # 0's k-major consumption order below)
            w_sb = wpool.tile([P, NVB, NK, P], DT, name="w_sb")
            for j in range(NCH0):
                nc.sync.dma_start(
                    out=w_sb[:, :, j * KC0 : (j + 1) * KC0],
                    in_=w_d[:, :, j * KC0 : (j + 1) * KC0],
                )

            # --- xt rowblock tiles (one tag, 3 bufs); r0 in chunks on the
            # scalar queue (parallel with the sync queue's w chunks); the
            # tiny tanh constants ride the scalar queue right after chunk 0
            # so the first tanh never waits
            def xt_tile(r):
                return xpool.tile(
                    [P, NK, RB], DT, name=f"xt_{r}", tag="xt", bufs=3
                )

            xt_cur = xt_tile(0)
            bct_sb = spool.tile([P, NVB], f32, name="bct_sb")
            vwt_sb = spool.tile([P, NVB], f32, name="vwt_sb")
            vwtr_sb = spool.tile([P, NVB], f32r, name="vwtr_sb")
            for j in range(NCH0):
                nc.scalar.dma_start(
                    out=xt_cur[:, j * KC0 : (j + 1) * KC0],
                    in_=xt_d[0, :, j * KC0 : (j + 1) * KC0],
                )
                if j == 0:
                    nc.scalar.dma_start(out=bct_sb[:], in_=bct_d[:, :])
                    nc.scalar.dma_start(out=vwt_sb[:], in_=vwt_d[:, :])
                    nc.scalar.dma_start(out=vwtr_sb[:], in_=vwtr_d[:, :])

            # stage-2 constant (first needed one full rowblock in)
            ones_sb = spool.tile([P, 1], f32r, name="ones_sb")
            nc.gpsimd.dma_start(out=ones_sb[:], in_=ones_d[:, :])

            def load_xt(r, t):
                for h in range(2):
                    nc.sync.dma_start(
                        out=t[:, h * KH : (h + 1) * KH],
                        in_=xt_d[r, :, h * KH : (h + 1) * KH],
                    )

            # xt r1 trails the w chunks on the sync queue (needed at ~40us)
            xt_nxt = xt_tile(1)
            load_xt(1, xt_nxt)

            # u lives in SBUF on partition 0 for the whole kernel (no DRAM
            # bounce; [1, ROWS] so all u_ps copies stay on partition 0)
            u2 = spool.tile([1, ROWS], f32, name="u2")
            # esum slots: [b0 total, b1 early part, b1 last chunk, b1 total]
            esum = spool.tile([1, 4], f32, name="esum")
            rec = spool.tile([1, BC], f32, name="rec")
            att_flat = att_d[:, :].rearrange("b s -> (b s)")

            def emit_softmax(b):
                # softmax over s for local batch b (u is tanh-bounded: no
                # max subtraction; the vb offset is softmax-invariant)
                nc.scalar.activation(
                    u2[:, b * S : (b + 1) * S],
                    u2[:, b * S : (b + 1) * S],
                    mybir.ActivationFunctionType.Exp,
                    accum_out=esum[:, b : b + 1],
                )
                nc.vector.reciprocal(rec[:, b : b + 1], esum[:, b : b + 1])
                nc.vector.tensor_scalar_mul(
                    u2[:, b * S : (b + 1) * S],
                    u2[:, b * S : (b + 1) * S],
                    rec[:, b : b + 1],
                )
                nc.sync.dma_start(
                    out=att_flat[b * S : (b + 1) * S],
                    in_=u2[:, b * S : (b + 1) * S],
                )

            pending = []  # deferred (emit_fn) for the previous rowblock
            LAST = NRB - 1

            def tanh_tt(ps, vb, tag="tt", bufs=3):
                tt = tpool.tile([P, RB], f32r, name="tt", tag=tag, bufs=bufs)
                nc.scalar.activation(
                    tt[:],
                    ps[:],
                    mybir.ActivationFunctionType.Tanh,
                    bias=bct_sb[:, vb : vb + 1],
                    scale=1.0,
                )
                return tt

            def acc_step(acc, tt, vb):
                if vb == 0:
                    nc.vector.tensor_scalar_mul(acc[:], tt[:], vwt_sb[:, 0:1])
                else:
                    nc.vector.scalar_tensor_tensor(
                        acc[:],
                        tt[:],
                        vwt_sb[:, vb : vb + 1],
                        acc[:],
                        op0=mybir.AluOpType.mult,
                        op1=mybir.AluOpType.add,
                    )

            PSB = 7  # "ps" ring depth: 7 banks + 1 for "ups" = all 8

            def ps_tile():
                return pspool.tile([P, RB], f32, name="ps", tag="ps", bufs=PSB)

            def make_stage2(r, acc):
                def fn():
                    u_ps = pspool.tile(
                        [1, RB], f32, name="u_ps", tag="ups", bufs=1
                    )
                    nc.tensor.matmul(
                        u_ps[:], ones_sb[:], acc[:], start=True, stop=True
                    )
                    nc.vector.tensor_copy(u2[:, r * RB : (r + 1) * RB], u_ps[:])

                return fn

            # ---- rowblock 0: k-major for vb0..6 across 7 PSUM banks, so
            # the PE tracks the k-chunk DMA arrival order and is compute-
            # bound from the first chunk on (w delivery is the startup
            # critical path); vb7 runs vb-major once all chunks are in
            ps0 = [ps_tile() for _ in range(PSB)]
            for j in range(NCH0):
                for vb in range(PSB):
                    for kk in range(KC0):
                        k = j * KC0 + kk
                        nc.tensor.matmul(
                            ps0[vb][:],
                            w_sb[:, vb, k],
                            xt_cur[:, k],
                            start=(j == 0 and kk == 0),
                            stop=(j == NCH0 - 1 and kk == KC0 - 1),
                        )
            t = xt_tile(2)
            load_xt(2, t)
            acc = tpool.tile([P, RB], f32r, name="acc", tag="acc", bufs=2)
            for vb in range(PSB):
                acc_step(acc, tanh_tt(ps0[vb], vb), vb)
            ps7 = ps_tile()
            for k in range(NK):
                nc.tensor.matmul(
                    ps7[:],
                    w_sb[:, PSB, k],
                    xt_cur[:, k],
                    start=(k == 0),
                    stop=(k == NK - 1),
                )
            acc_step(acc, tanh_tt(ps7, PSB), PSB)
            pending = [make_stage2(0, acc)]
            xt_cur, xt_nxt = xt_nxt, t

            # ---- rowblocks 1..NRB-2: vb-major, 3-deep psum pipeline ----
            for r in range(1, LAST):
                acc = tpool.tile([P, RB], f32r, name="acc", tag="acc", bufs=2)
                for vb in range(NVB):
                    ps = ps_tile()
                    for k in range(NK):
                        nc.tensor.matmul(
                            ps[:],
                            w_sb[:, vb, k],
                            xt_cur[:, k],
                            start=(k == 0),
                            stop=(k == NK - 1),
                        )
                    if vb == 0:
                        # prefetch r+2 while vb0 streams
                        if r + 2 < NRB:
                            t = xt_tile(r + 2)
                            load_xt(r + 2, t)
                        else:
                            t = None
                        # emit the previous rowblock's partition-sum matmul
                        # here: its DVE inputs are long since ready, and the
                        # PE has 32 matmuls of slack queued ahead of it
                        for fn in pending:
                            fn()
                        pending = []
                        if r == RB_PER_B:
                            # batch 0's u is complete: run its softmax and
                            # output DMA under the batch-1 compute stream
                            emit_softmax(0)
                    acc_step(acc, tanh_tt(ps, vb), vb)
                pending = [make_stage2(r, acc)]
                xt_cur, xt_nxt = xt_nxt, t

            # ---- last rowblock: stage-2 back on the PE (vwt-stationary,
            # deferred one vb) for the shortest possible tail chain ----
            u_ps = pspool.tile([1, RB], f32, name="u_ps15", tag="ups", bufs=1)
            tts = []
            for vb in range(NVB):
                ps = ps_tile()
                for k in range(NK):
                    nc.tensor.matmul(
                        ps[:],
                        w_sb[:, vb, k],
                        xt_cur[:, k],
                        start=(k == 0),
                        stop=(k == NK - 1),
                    )
                if vb == 0:
                    for fn in pending:
                        fn()
                    pending = []
                    # early exp over batch-1 rowblocks 8..14 (all copied)
                    nc.scalar.activation(
                        u2[:, S : S + (RB_PER_B - 1) * RB],
                        u2[:, S : S + (RB_PER_B - 1) * RB],
                        mybir.ActivationFunctionType.Exp,
                        accum_out=esum[:, 1:2],
                    )
                else:
                    nc.tensor.matmul(
                        u_ps[:],
                        vwtr_sb[:, vb - 1 : vb],
                        tts[vb - 1][:],
                        start=(vb == 1),
                        stop=False,
                    )
                tts.append(tanh_tt(ps, vb, tag="tt15", bufs=NVB))
            nc.tensor.matmul(
                u_ps[:],
                vwtr_sb[:, NVB - 1 : NVB],
                tts[NVB - 1][:],
                start=False,
                stop=True,
            )
            nc.vector.tensor_copy(u2[:, LAST * RB : (LAST + 1) * RB], u_ps[:])

            # final exp chunk (rowblock 15), combine sums, scale, ship out
            nc.scalar.activation(
                u2[:, S + (RB_PER_B - 1) * RB : 2 * S],
                u2[:, S + (RB_PER_B - 1) * RB : 2 * S],
                mybir.ActivationFunctionType.Exp,
                accum_out=esum[:, 2:3],
            )
            nc.vector.tensor_tensor(
                esum[:, 3:4], esum[:, 1:2], esum[:, 2:3], op=mybir.AluOpType.add
            )
            nc.vector.reciprocal(rec[:, 1:2], esum[:, 3:4])
            nc.vector.tensor_scalar_mul(
                u2[:, S : 2 * S], u2[:, S : 2 * S], rec[:, 1:2]
            )
            nc.sync.dma_start(out=att_flat[S : 2 * S], in_=u2[:, S : 2 * S])

    _split_multiwait(nc)
    return nc


# ---------------------------------------------------------------------------
# Host entry point
# ---------------------------------------------------------------------------
def kernel(hidden, z, Ww, bw, Wz, bz, Vw, vb, w_a):
    _install_drain_patch()
    from concourse.bass_utils import run_bass_kernel_spmd

    np_main = ml_dtypes.bfloat16 if MAIN_DT == "bf16" else np.float32

    # ---- host-side shard prep ----
    hid_t = np.ascontiguousarray(
        np.asarray(hidden).astype(np_main).transpose(2, 1, 0)
    )  # [H2, B, S]
    z_t = np.ascontiguousarray(
        np.asarray(z).astype(np_main).transpose(2, 1, 0)
    )  # [H2, B, S]

    w_cat = np.concatenate(
        [np.asarray(Ww), np.asarray(Wz)], axis=0
    ).astype(np_main)  # [H, V]
    # reorder to the SBUF slab layout: w_r[p, vb, k, q] = W[k*P+p, vb*P+q]
    w_r = np.ascontiguousarray(
        w_cat.reshape(NK, P, NVB, P).transpose(1, 2, 0, 3)
    )

    bias = (
        np.asarray(bw).astype(np.float64)
        + np.asarray(bz).astype(np.float64)
        + float(np.asarray(w_a)) * ALPHA_S
    ).astype(np.float32)  # [V]
    bct = np.ascontiguousarray(bias.reshape(NVB, P).T)  # [P, NVB]
    vwt = np.ascontiguousarray(
        np.asarray(Vw).astype(np.float32).reshape(NVB, P).T
    )  # [P, NVB]

    in_maps = []
    for c in range(NCORES):
        xt_c = np.empty((H, ROWS), dtype=np_main)
        xt_c[:H2] = hid_t[:, 2 * c : 2 * c + 2, :].reshape(H2, ROWS)
        xt_c[H2:] = z_t[:, 2 * c : 2 * c + 2, :].reshape(H2, ROWS)
        # pre-tile: xt_pre[r, p, k, c] = X^T[k*P+p, r*RB+c]
        xt_pre = np.ascontiguousarray(
            xt_c.reshape(NK, P, NRB, RB).transpose(2, 1, 0, 3)
        )
        in_maps.append(
            {
                "xt": xt_pre,
                "w": w_r,
                "bct": bct,
                "vwt": vwt,
                "vwtr": vwt,
                "ones": np.ones((P, 1), dtype=np.float32),
            }
        )

    nc = _build_nc()
    res = run_bass_kernel_spmd(nc, in_maps, list(range(NCORES)))

    out = np.empty((S, B, 1), dtype=np.float32)
    for c in range(NCORES):
        att = res.results[c]["att"]  # [BC, S]
        for b in range(BC):
            out[:, 2 * c + b, 0] = att[b]
    return out
